# revision 19
# baseline (speedup 1.0000x reference)
"""Trainium2 Bass kernel for nn_Mask_58351425683882.

Computes out = (x * mask) @ from_to with
  x:      [16, 8192]  f32
  mask:   [8192]      f32 (0/1)
  from_to:[8192,8192] f32 (one-hot permutation columns)

from_to is fully determined by mask: out[:, cumsum(mask)[i]-1] = x[:, i]
for every i with mask[i]==1, and out[:, n1:] = 0 (n1 = popcount). The
baseline's dense [8192, 8192] matmul (256MB of HBM traffic, ~104us at
the DMA roofline) collapses to a ~1MB masked compaction.

Per-core (8 cores, 1024 source columns each; partition p owns the 8
consecutive sources 1024c + 8p + j):
  1. DVE computes the within-partition exclusive rank lr[p,j] with a
     free-axis prefix scan of the mask slice.
  2. PE computes each partition's global destination base (exclusive
     cumsum of per-partition counts + prefix of preceding cores) with
     three tiny matmuls (per-128-block sums; triangular-ones cumsum;
     static core-selector), DVE copies block sums PSUM->SBUF between.
  3. dest[p,j] = base[p] + lr[p,j], +1e6 for mask==0 (out of bounds).
  4. GPSIMD issues 8 indirect SWDGE scatters, one per source slot j:
     each moves partition p's 64B column j of x^T to out^T row
     dest[p,j]; OOB rows are silently dropped. Slot j's live
     destinations strictly increase in p and slots are globally
     disjoint, so writes never collide and need no ordering.
  5. The SP HWDGE queue zero-fills out^T concurrently (FIFO-ordered
     behind the input load so input completion sems are not delayed).
Host sums the 8 disjoint per-core results (zeros elsewhere) and
transposes. Values pass through unmodified -> bit-exact result.

HW findings baked in (all verified on TRN2 via probes):
 - indirect DMA consumes ONE offset per partition and moves the full
   contiguous SBUF row per descriptor; multi-offset free dims ignored.
 - compute_op (CCE accumulate) on indirect DMA is ignored -> the
   slot-class decomposition below avoids any overlapping writes.
 - every gpsimd DMA instruction costs ~1us of SWDGE descriptor
   generation; regular DMAs go on SP/Act HWDGE queues instead.
 - the DVE has no intra-engine RAW interlock for short ops: dependent
   DVE pairs are ordered through the vch semaphore.
 - f32->int32 tensor_copy (bypass ALU) corrupts data; a non-bypass ALU
   op (add) engages the output converter correctly.
"""

import sys

for _p in ("/opt/trn_rl_repo",):
    if _p not in sys.path:
        sys.path.insert(0, _p)

import numpy as np

import concourse.bass as bass
import concourse.mybir as mybir
from concourse.bass_utils import run_bass_kernel_spmd

B = 16          # batch rows of x
N = 8192        # feature dim
NCORES = 8
P = 128         # SBUF partitions
KT = N // P     # 64 mask blocks of 128
JB = 8          # sources per partition

_F32 = mybir.dt.float32
_I32 = mybir.dt.int32

# "rest" input blob column layout (f32, [128, REST_W])
_C_MK = 0                  # [:, 0:64]    mask, mk[p,k] = mask[k*128+p]
_C_ONE = _C_MK + KT        # [:, 64:65]   ones column
_C_TRI = _C_ONE + 1        # [:, 65:193]  triu1[k,m]=1 if k<m
_C_SELB = _C_TRI + P       # [:, 193:321] selbc[k,m]=1 if k<8c (rows 0..63)
_C_XT = _C_SELB + P        # [:, 321:449] xt[p, j*16+b] = x[b, 1024c+8p+j]
REST_W = _C_XT + JB * B    # 449

NP = 16512   # out^T rows incl. trash region for mask==0 writes
OOB = 8192.0  # mask==0 dests land in rows [8192, 16400): never read,
              # collisions harmless -> no bounds-check register needed


def build_nc():
    nc = bass.Bass()

    mo = nc.dram_tensor("mo", [P, JB], _F32, kind="ExternalInput")
    rest = nc.dram_tensor("rest", [P, REST_W], _F32, kind="ExternalInput")
    outT = nc.dram_tensor("outT", [NP, B], _F32, kind="ExternalOutput")

    from contextlib import ExitStack

    with ExitStack() as ctx:
        mo_sem = ctx.enter_context(nc.semaphore("mo_sem"))
        rest_sem = ctx.enter_context(nc.semaphore("rest_sem"))
        zm_sem = ctx.enter_context(nc.semaphore("zm_sem"))
        z_sem = ctx.enter_context(nc.semaphore("z_sem"))
        vch = ctx.enter_context(nc.semaphore("vch"))
        pe_bs_sem = ctx.enter_context(nc.semaphore("pe_bs_sem"))
        pe_base_sem = ctx.enter_context(nc.semaphore("pe_base_sem"))
        dest_sem = ctx.enter_context(nc.semaphore("dest_sem"))
        sc_sem = ctx.enter_context(nc.semaphore("sc_sem"))

        mo_sb = ctx.enter_context(nc.sbuf_tensor("mo_sb", [P, JB], _F32))
        rest_sb = ctx.enter_context(nc.sbuf_tensor("rest_sb", [P, REST_W], _F32))
        zeros = ctx.enter_context(nc.sbuf_tensor("zeros", [P, N * B // P], _F32))
        incl = ctx.enter_context(nc.sbuf_tensor("incl", [P, JB], _F32))
        lr = ctx.enter_context(nc.sbuf_tensor("lr", [P, JB], _F32))
        bs_sb = ctx.enter_context(nc.sbuf_tensor("bs_sb", [KT, 1], _F32))
        t1 = ctx.enter_context(nc.sbuf_tensor("t1", [P, JB], _F32))
        o3 = ctx.enter_context(nc.sbuf_tensor("o3", [P, JB], _F32))
        dest_i = ctx.enter_context(nc.sbuf_tensor("dest_i", [P, JB], _I32))
        ps_bs = ctx.enter_context(nc.psum_tensor("ps_bs", [KT, 1], _F32))
        ps_base = ctx.enter_context(nc.psum_tensor("ps_base", [P, 1], _F32))
        warm_sb = ctx.enter_context(nc.sbuf_tensor("warm_sb", [1, 4], _F32))
        warm_i = ctx.enter_context(nc.sbuf_tensor("warm_i", [P, 1], _I32))
        w2_sem = ctx.enter_context(nc.semaphore("w2_sem"))
        block = ctx.enter_context(nc.Block(no_gpsimd_drain=True))

        # real out^T rows viewed as [128, 1024] for the two zero-fill halves
        outz = outT[:N, :].rearrange("(a b) c -> a (b c)", a=P)

        @block.sync
        def _(sync):
            # mo first: its 4KB lands crisply before rest's descriptors
            # occupy the engines, starting the DVE scan chain earlier.
            # z-fill halves ride the same HWDGE queues behind the inputs:
            # per-queue FIFO keeps their descriptors behind the inputs', so
            # input completion sems are not delayed by the zero burst
            # (v3 regression, trace-verified)
            sync.dma_start(mo_sb[:, :], mo[:, :]).then_inc(mo_sem, 16)

        @block.scalar
        def _(scalar):
            # rest lands early on the otherwise-idle Act queue; the whole
            # 512KB z-fill rides FIFO behind it, starting its DGE ~0.7us
            # sooner than splitting across queues (z gated the scatters)
            scalar.dma_start(rest_sb[:, :], rest[:, :]).then_inc(rest_sem, 16)
            scalar.wait_ge(zm_sem, 1)
            scalar.dma_start(outz[:, :], zeros[:, :]).then_inc(z_sem, 16)

        @block.vector
        def _(vector):
            # static trash offsets for the indirect-path pre-warm scatter
            vector.memset(warm_i[:, :], 16000).then_inc(w2_sem, 1)
            vector.memset(zeros[:, :], 0.0).then_inc(zm_sem, 1)
            vector.wait_ge(mo_sem, 16)
            # A = OOB - (OOB+1)*mo  (folds rank subtract + OOB select:
            # dest = incl + A + base); independent of the scan
            vector.tensor_scalar(
                o3[:, :], mo_sb[:, :], -(OOB + 1.0), OOB,
                mybir.AluOpType.mult, mybir.AluOpType.add,
            ).then_inc(vch, 1)
            # incl[p,j] = sum_{j'<=j} mo[p,j']
            vector.tensor_tensor_scan(
                incl[:, :], mo_sb[:, :], mo_sb[:, :], 0.0,
                mybir.AluOpType.add, mybir.AluOpType.bypass,
            ).then_inc(vch, 1)
            # block sums PSUM->SBUF for the selector matmul (on DVE: the
            # Act engine's first op pays a 1.28us table load, trace-verified);
            # ordered before t1 because it gates the pe_base matmul chain
            vector.wait_ge(pe_bs_sem, 1)
            vector.tensor_copy(bs_sb[:, :], ps_bs[:, :]).then_inc(vch, 1)
            vector.wait_ge(vch, 2)
            vector.tensor_tensor(
                t1[:, :], incl[:, :], o3[:, :], mybir.AluOpType.add
            ).then_inc(vch, 1)
            vector.wait_ge(pe_base_sem, 1)
            vector.wait_ge(vch, 4)
            # int32 out via non-bypass ALU -> correct converter
            vector.tensor_tensor(
                dest_i[:, :], t1[:, :],
                ps_base[:, 0:1].broadcast_to([P, JB]),
                mybir.AluOpType.add,
            ).then_inc(dest_sem, 1)

        @block.tensor
        def _(tensor):
            tensor.wait_ge(rest_sem, 16)
            # per-block mask sums: ps_bs[k] = sum_p mask[k*128+p]
            tensor.matmul(
                ps_bs[:, :],
                rest_sb[:, _C_MK:_C_MK + KT],
                rest_sb[:, _C_ONE:_C_ONE + 1],
                start=True,
                stop=True,
            ).then_inc(pe_bs_sem, 1)
            # base[p] = sum_{p'<p} cnt[p'] + sum_{k<8c} bs[k]
            tensor.wait_ge(vch, 2)
            tensor.matmul(
                ps_base[:, :],
                rest_sb[:, _C_TRI:_C_TRI + P],
                incl[:, JB - 1:JB],
                start=True,
                stop=False,
            )
            tensor.wait_ge(vch, 3)
            tensor.matmul(
                ps_base[:, :],
                rest_sb[:KT, _C_SELB:_C_SELB + P],
                bs_sb[:, :],
                start=False,
                stop=True,
            ).then_inc(pe_base_sem, 1)

        @block.gpsimd
        def _(gpsimd):
            # pre-warm the SWDGE path (first Q7 DMA instruction pays extra
            # init, trace-verified ~1us) while inputs are still in flight;
            # the indirect desc-gen path is warmed separately with a scatter
            # of garbage into the never-read trash rows (collisions harmless)
            gpsimd.dma_start(warm_sb[:, :], rest[0:1, 0:4]).then_inc(sc_sem, 16)
            gpsimd.wait_ge(w2_sem, 1)
            gpsimd.indirect_dma_start(
                out=outT[:, :],
                out_offset=bass.IndirectOffsetOnAxis(ap=warm_i[:, :], axis=0),
                in_=zeros[:, 0:B],
                in_offset=None,
            ).then_inc(sc_sem, 16)
            gpsimd.wait_ge(z_sem, 16)
            gpsimd.wait_ge(dest_sem, 1)
            for j in range(JB):
                gpsimd.indirect_dma_start(
                    out=outT[:, :],
                    out_offset=bass.IndirectOffsetOnAxis(
                        ap=dest_i[:, j:j + 1], axis=0
                    ),
                    in_=rest_sb[:, _C_XT + j * B:_C_XT + (j + 1) * B],
                    in_offset=None,
                ).then_inc(sc_sem, 16)
            gpsimd.wait_ge(sc_sem, 16 * (JB + 2))

    return nc


def _prepare_in_maps(x, mask, from_to):
    x = np.asarray(x, dtype=np.float32)
    mask = np.asarray(mask, dtype=np.float32)

    mk = np.ascontiguousarray(mask.reshape(KT, P).T)          # [128, 64]
    ones = np.ones((P, 1), dtype=np.float32)
    triu1 = np.triu(np.ones((P, P), dtype=np.float32), 1)

    in_maps = []
    for c in range(NCORES):
        mo = np.ascontiguousarray(mask.reshape(NCORES, P, JB)[c])
        selbc = np.zeros((P, P), dtype=np.float32)
        selbc[:JB * c, :] = 1.0
        xt = x.reshape(B, NCORES, P, JB)[:, c].transpose(1, 2, 0)  # [128,8,16]
        xt = np.ascontiguousarray(xt.reshape(P, JB * B))
        rest = np.concatenate([mk, ones, triu1, selbc, xt], axis=1)
        in_maps.append({"mo": mo, "rest": np.ascontiguousarray(rest)})
    return in_maps


def _run(x, mask, from_to, trace=False):
    nc = build_nc()
    in_maps = _prepare_in_maps(x, mask, from_to)
    res = run_bass_kernel_spmd(nc, in_maps, core_ids=list(range(NCORES)), trace=trace)
    acc = res.results[0]["outT"][:N].astype(np.float32)
    for c in range(1, NCORES):
        acc = acc + res.results[c]["outT"][:N]
    return np.ascontiguousarray(acc.T), res


def kernel(x, mask, from_to):
    out, _ = _run(x, mask, from_to, trace=False)
    return out


# revision 20
# speedup vs baseline: 1.0073x; 1.0073x over previous
"""Trainium2 Bass kernel for nn_Mask_58351425683882.

Computes out = (x * mask) @ from_to with
  x:      [16, 8192]  f32
  mask:   [8192]      f32 (0/1)
  from_to:[8192,8192] f32 (one-hot permutation columns)

from_to is fully determined by mask: out[:, cumsum(mask)[i]-1] = x[:, i]
for every i with mask[i]==1, and out[:, n1:] = 0 (n1 = popcount). The
baseline's dense [8192, 8192] matmul (256MB of HBM traffic, ~104us at
the DMA roofline) collapses to a ~1MB masked compaction.

Per-core (8 cores, 1024 source columns each; partition p owns the 8
consecutive sources 1024c + 8p + j):
  1. DVE computes the within-partition exclusive rank lr[p,j] with a
     free-axis prefix scan of the mask slice.
  2. PE computes each partition's global destination base (exclusive
     cumsum of per-partition counts + prefix of preceding cores) with
     three tiny matmuls (per-128-block sums; triangular-ones cumsum;
     static core-selector), DVE copies block sums PSUM->SBUF between.
  3. dest[p,j] = base[p] + lr[p,j], +1e6 for mask==0 (out of bounds).
  4. GPSIMD issues 8 indirect SWDGE scatters, one per source slot j:
     each moves partition p's 64B column j of x^T to out^T row
     dest[p,j]; OOB rows are silently dropped. Slot j's live
     destinations strictly increase in p and slots are globally
     disjoint, so writes never collide and need no ordering.
  5. The SP HWDGE queue zero-fills out^T concurrently (FIFO-ordered
     behind the input load so input completion sems are not delayed).
Host sums the 8 disjoint per-core results (zeros elsewhere) and
transposes. Values pass through unmodified -> bit-exact result.

HW findings baked in (all verified on TRN2 via probes):
 - indirect DMA consumes ONE offset per partition and moves the full
   contiguous SBUF row per descriptor; multi-offset free dims ignored.
 - compute_op (CCE accumulate) on indirect DMA is ignored -> the
   slot-class decomposition below avoids any overlapping writes.
 - every gpsimd DMA instruction costs ~1us of SWDGE descriptor
   generation; regular DMAs go on SP/Act HWDGE queues instead.
 - the DVE has no intra-engine RAW interlock for short ops: dependent
   DVE pairs are ordered through the vch semaphore.
 - f32->int32 tensor_copy (bypass ALU) corrupts data; a non-bypass ALU
   op (add) engages the output converter correctly.
"""

import sys

for _p in ("/opt/trn_rl_repo",):
    if _p not in sys.path:
        sys.path.insert(0, _p)

import numpy as np

import concourse.bass as bass
import concourse.mybir as mybir
from concourse.bass_utils import run_bass_kernel_spmd

B = 16          # batch rows of x
N = 8192        # feature dim
NCORES = 8
P = 128         # SBUF partitions
KT = N // P     # 64 mask blocks of 128
JB = 8          # sources per partition

_F32 = mybir.dt.float32
_I32 = mybir.dt.int32

# "rest" input blob column layout (f32, [128, REST_W])
_C_MK = 0                  # [:, 0:64]    mask, mk[p,k] = mask[k*128+p]
_C_ONE = _C_MK + KT        # [:, 64:65]   ones column
_C_TRI = _C_ONE + 1        # [:, 65:193]  triu1[k,m]=1 if k<m
_C_SELB = _C_TRI + P       # [:, 193:321] selbc[k,m]=1 if k<8c (rows 0..63)
_C_XT = _C_SELB + P        # [:, 321:449] xt[p, j*16+b] = x[b, 1024c+8p+j]
REST_W = _C_XT + JB * B    # 449

NP = 16512   # out^T rows incl. trash region for mask==0 writes
OOB = 8192.0  # mask==0 dests land in rows [8192, 16400): never read,
              # collisions harmless -> no bounds-check register needed


def build_nc():
    nc = bass.Bass()

    mo = nc.dram_tensor("mo", [P, JB], _F32, kind="ExternalInput")
    rest = nc.dram_tensor("rest", [P, REST_W], _F32, kind="ExternalInput")
    outT = nc.dram_tensor("outT", [NP, B], _F32, kind="ExternalOutput")

    from contextlib import ExitStack

    with ExitStack() as ctx:
        mo_sem = ctx.enter_context(nc.semaphore("mo_sem"))
        rest_sem = ctx.enter_context(nc.semaphore("rest_sem"))
        zm_sem = ctx.enter_context(nc.semaphore("zm_sem"))
        z_sem = ctx.enter_context(nc.semaphore("z_sem"))
        vch = ctx.enter_context(nc.semaphore("vch"))
        pe_bs_sem = ctx.enter_context(nc.semaphore("pe_bs_sem"))
        pe_base_sem = ctx.enter_context(nc.semaphore("pe_base_sem"))
        dest_sem = ctx.enter_context(nc.semaphore("dest_sem"))
        sc_sem = ctx.enter_context(nc.semaphore("sc_sem"))

        mo_sb = ctx.enter_context(nc.sbuf_tensor("mo_sb", [P, JB], _F32))
        rest_sb = ctx.enter_context(nc.sbuf_tensor("rest_sb", [P, REST_W], _F32))
        zeros = ctx.enter_context(nc.sbuf_tensor("zeros", [P, N * B // P], _F32))
        incl = ctx.enter_context(nc.sbuf_tensor("incl", [P, JB], _F32))
        lr = ctx.enter_context(nc.sbuf_tensor("lr", [P, JB], _F32))
        bs_sb = ctx.enter_context(nc.sbuf_tensor("bs_sb", [KT, 1], _F32))
        t1 = ctx.enter_context(nc.sbuf_tensor("t1", [P, JB], _F32))
        o3 = ctx.enter_context(nc.sbuf_tensor("o3", [P, JB], _F32))
        dest_i = ctx.enter_context(nc.sbuf_tensor("dest_i", [P, JB], _I32))
        ps_bs = ctx.enter_context(nc.psum_tensor("ps_bs", [KT, 1], _F32))
        ps_base = ctx.enter_context(nc.psum_tensor("ps_base", [P, 1], _F32))
        warm_sb = ctx.enter_context(nc.sbuf_tensor("warm_sb", [1, 4], _F32))
        warm_i = ctx.enter_context(nc.sbuf_tensor("warm_i", [P, 1], _I32))
        w2_sem = ctx.enter_context(nc.semaphore("w2_sem"))
        block = ctx.enter_context(nc.Block(no_gpsimd_drain=True))

        # real out^T rows viewed as [128, 1024] for the two zero-fill halves
        outz = outT[:N, :].rearrange("(a b) c -> a (b c)", a=P)

        @block.sync
        def _(sync):
            # mo first: its 4KB lands crisply before rest's descriptors
            # occupy the engines, starting the DVE scan chain earlier.
            # z-fill halves ride the same HWDGE queues behind the inputs:
            # per-queue FIFO keeps their descriptors behind the inputs', so
            # input completion sems are not delayed by the zero burst
            # (v3 regression, trace-verified)
            sync.dma_start(mo_sb[:, :], mo[:, :]).then_inc(mo_sem, 16)
            sync.wait_ge(zm_sem, 1)
            sync.dma_start(outz[:, 512:], zeros[:, 512:]).then_inc(z_sem, 16)

        @block.scalar
        def _(scalar):
            scalar.dma_start(rest_sb[:, :], rest[:, :]).then_inc(rest_sem, 16)
            scalar.wait_ge(zm_sem, 1)
            scalar.dma_start(outz[:, :512], zeros[:, :512]).then_inc(z_sem, 16)

        @block.vector
        def _(vector):
            # static trash offsets for the indirect-path pre-warm scatter
            vector.memset(warm_i[:, :], 16000).then_inc(w2_sem, 1)
            vector.memset(zeros[:, :], 0.0).then_inc(zm_sem, 1)
            vector.wait_ge(mo_sem, 16)
            # A = OOB - (OOB+1)*mo  (folds rank subtract + OOB select:
            # dest = incl + A + base); independent of the scan
            vector.tensor_scalar(
                o3[:, :], mo_sb[:, :], -(OOB + 1.0), OOB,
                mybir.AluOpType.mult, mybir.AluOpType.add,
            ).then_inc(vch, 1)
            # incl[p,j] = sum_{j'<=j} mo[p,j']
            vector.tensor_tensor_scan(
                incl[:, :], mo_sb[:, :], mo_sb[:, :], 0.0,
                mybir.AluOpType.add, mybir.AluOpType.bypass,
            ).then_inc(vch, 1)
            # block sums PSUM->SBUF for the selector matmul (on DVE: the
            # Act engine's first op pays a 1.28us table load, trace-verified);
            # ordered before t1 because it gates the pe_base matmul chain
            vector.wait_ge(pe_bs_sem, 1)
            vector.tensor_copy(bs_sb[:, :], ps_bs[:, :]).then_inc(vch, 1)
            vector.wait_ge(vch, 2)
            vector.tensor_tensor(
                t1[:, :], incl[:, :], o3[:, :], mybir.AluOpType.add
            ).then_inc(vch, 1)
            vector.wait_ge(pe_base_sem, 1)
            vector.wait_ge(vch, 4)
            # int32 out via non-bypass ALU -> correct converter
            vector.tensor_tensor(
                dest_i[:, :], t1[:, :],
                ps_base[:, 0:1].broadcast_to([P, JB]),
                mybir.AluOpType.add,
            ).then_inc(dest_sem, 1)

        @block.tensor
        def _(tensor):
            tensor.wait_ge(rest_sem, 16)
            # per-block mask sums: ps_bs[k] = sum_p mask[k*128+p]
            tensor.matmul(
                ps_bs[:, :],
                rest_sb[:, _C_MK:_C_MK + KT],
                rest_sb[:, _C_ONE:_C_ONE + 1],
                start=True,
                stop=True,
            ).then_inc(pe_bs_sem, 1)
            # base[p] = sum_{p'<p} cnt[p'] + sum_{k<8c} bs[k]
            tensor.wait_ge(vch, 2)
            tensor.matmul(
                ps_base[:, :],
                rest_sb[:, _C_TRI:_C_TRI + P],
                incl[:, JB - 1:JB],
                start=True,
                stop=False,
            )
            tensor.wait_ge(vch, 3)
            tensor.matmul(
                ps_base[:, :],
                rest_sb[:KT, _C_SELB:_C_SELB + P],
                bs_sb[:, :],
                start=False,
                stop=True,
            ).then_inc(pe_base_sem, 1)

        @block.gpsimd
        def _(gpsimd):
            # pre-warm the SWDGE path (first Q7 DMA instruction pays extra
            # init, trace-verified ~1us) while inputs are still in flight;
            # the indirect desc-gen path is warmed separately with a scatter
            # of garbage into the never-read trash rows (collisions harmless)
            gpsimd.dma_start(warm_sb[:, :], rest[0:1, 0:4]).then_inc(sc_sem, 16)
            gpsimd.wait_ge(w2_sem, 1)
            gpsimd.indirect_dma_start(
                out=outT[:, :],
                out_offset=bass.IndirectOffsetOnAxis(ap=warm_i[:, :], axis=0),
                in_=zeros[:, 0:B],
                in_offset=None,
            ).then_inc(sc_sem, 16)
            gpsimd.wait_ge(z_sem, 32)
            gpsimd.wait_ge(dest_sem, 1)
            for j in range(JB):
                gpsimd.indirect_dma_start(
                    out=outT[:, :],
                    out_offset=bass.IndirectOffsetOnAxis(
                        ap=dest_i[:, j:j + 1], axis=0
                    ),
                    in_=rest_sb[:, _C_XT + j * B:_C_XT + (j + 1) * B],
                    in_offset=None,
                ).then_inc(sc_sem, 16)
            gpsimd.wait_ge(sc_sem, 16 * (JB + 2))

    return nc


def _prepare_in_maps(x, mask, from_to):
    x = np.asarray(x, dtype=np.float32)
    mask = np.asarray(mask, dtype=np.float32)

    mk = np.ascontiguousarray(mask.reshape(KT, P).T)          # [128, 64]
    ones = np.ones((P, 1), dtype=np.float32)
    triu1 = np.triu(np.ones((P, P), dtype=np.float32), 1)

    in_maps = []
    for c in range(NCORES):
        mo = np.ascontiguousarray(mask.reshape(NCORES, P, JB)[c])
        selbc = np.zeros((P, P), dtype=np.float32)
        selbc[:JB * c, :] = 1.0
        xt = x.reshape(B, NCORES, P, JB)[:, c].transpose(1, 2, 0)  # [128,8,16]
        xt = np.ascontiguousarray(xt.reshape(P, JB * B))
        rest = np.concatenate([mk, ones, triu1, selbc, xt], axis=1)
        in_maps.append({"mo": mo, "rest": np.ascontiguousarray(rest)})
    return in_maps


def _run(x, mask, from_to, trace=False):
    nc = build_nc()
    in_maps = _prepare_in_maps(x, mask, from_to)
    res = run_bass_kernel_spmd(nc, in_maps, core_ids=list(range(NCORES)), trace=trace)
    acc = res.results[0]["outT"][:N].astype(np.float32)
    for c in range(1, NCORES):
        acc = acc + res.results[c]["outT"][:N]
    return np.ascontiguousarray(acc.T), res


def kernel(x, mask, from_to):
    out, _ = _run(x, mask, from_to, trace=False)
    return out
